# revision 56
# baseline (speedup 1.0000x reference)
"""Chunked attention kernel for Trainium2 (Bass/Tile), SPMD over 8 NeuronCores.

Problem (hardcoded):
  x: [B=8, C=1024, L=4096] fp32, Wq/Wk/Wv/Wo: [1024,1024] fp32 (stored [in,out]),
  biases [1024] fp32.  H=8 heads, head_dim=128, CHUNK=64 (block-diagonal attention).
  out = transpose(softmax((xt@Wq)(xt@Wk)^T/sqrt(128) blockwise) @ (xt@Wv) @ Wo, [B,C,L])

Sharding: data-parallel over B - one batch per core. No collectives.

Numerics: the four C x C projections run as fp8(e4m3) DoubleRow matmuls.
DoubleRow packs TWO 128-deep contraction tiles per PE instruction at 0.5
cycles/output-row. To keep accuracy, every projection input is split hi/lo
into two e4m3 tensors (value = hi + lo, representation error ~1.3e-3) and
each projection computes the three significant bilinear terms
hi*Whi + lo*Whi + hi*Wlo (the lo*Wlo term is ~1e-3 relative and dropped):
12 DoubleRow ops per [128,512] output tile = 3072 PE cycles vs 4096 for
fp16. The V projection additionally drops the LAST j-pair (k-tiles 6,7)
of its hi*Wlo correction: each dropped k-tile residual is ~2.6%/sqrt(8),
lifting total rel err from 2.3e-3 to a measured 1.35e-2 (vs the 2e-2
gate) and saving 2048 PE cycles per strip. Tensors are pre-scaled (x*16,
W*256, P*16) so e4m3's 3-bit mantissa sees values inside its normal
range; descales fold into eviction copies.

Engine balance: softmax denominators (replicated column sums of exp) run
on the Pool engine via partition_all_reduce (no PE ones-matmul); Q/K/V
psum evictions run on ACT (activation Copy with scale); reciprocal,
normalize and o_t evictions run on DVE. The P hi/lo split is two ops
straight off PSUM on separate engines: ph8 = fp8(SP*P) on ACT and
pl8 = (SP*P) - ph8 in one DVE scalar_tensor_tensor, so the output
projection never stalls behind a serial split chain. (GPSIMD cannot
read PSUM - the BIR verifier rejects it; CoreSim does not.)

Pipeline (per strip of 512 tokens): Q/K projections per head with the
scores/exp/denominator chain pipelined 3 heads behind; the previous
strip's output projection follows the QK loop (covering the tail of the
softmax chains); then the V projection; then PV + P split. x for the
next strip is prefetched (one batched DMA per tensor) before the output
DMAs are enqueued so the next strip never waits on the DMA queue.
Strip 0 is DMA-paced, so its Q/K work is k-blocked 4 heads at a time
(term-major, j-pair inner) to track the just-in-time startup DMA order;
the final strip runs V/PV/splits BEFORE the previous strip's output
projection so every split hides under that pure-runway PE work.
"""

import numpy as np
from contextlib import ExitStack

import concourse.bass as bass
import concourse.bacc as bacc
import concourse.tile as tile
import concourse.mybir as mybir
from concourse import bass_isa

B, C, L = 8, 1024, 4096
H, HD, CHUNK, PAIR = 8, 128, 64, 128
N_CORES = 8
KT = C // 128          # 8 contraction tiles
NJP = KT // 2          # DoubleRow j-pairs per term
LT = 512               # tokens per strip
F8 = mybir.dt.float8e4
F16 = mybir.dt.float16
F32 = mybir.dt.float32
NP8 = mybir.dt.np(F8)
SCALE = 1.0 / float(np.sqrt(HD))
DRMODE = mybir.MatmulPerfMode.DoubleRow
ACOPY = mybir.ActivationFunctionType.Copy
SX = 16.0              # x pre-scale into e4m3 range
SW = 256.0             # weight pre-scale (sigma 1/32 -> 8)
SP = 16.0              # attention-output pre-scale
QSCALE = 1.0 / (SX * SW)   # Q/K/V eviction descale
OSCALE = 1.0 / (SP * SW)   # out eviction descale
WNAMES = ("wq", "wk", "wv", "wo")


def _emit(ctx, tc, xh_d, xl_d, w_d, o_d, l_total):
    nc = tc.nc
    NS = l_total // LT     # strips
    NP = LT // PAIR        # chunk-pairs (= token 128-tiles) per strip

    wpool = ctx.enter_context(tc.tile_pool(name="w", bufs=1))
    xpool = ctx.enter_context(tc.tile_pool(name="xp", bufs=2))
    qpool = ctx.enter_context(tc.tile_pool(name="qp", bufs=1))
    vpool = ctx.enter_context(tc.tile_pool(name="vp", bufs=2))
    epool = ctx.enter_context(tc.tile_pool(name="ep", bufs=1))
    rpool = ctx.enter_context(tc.tile_pool(name="rp", bufs=2))
    dpool = ctx.enter_context(tc.tile_pool(name="dp", bufs=2))
    npool = ctx.enter_context(tc.tile_pool(name="np", bufs=2))
    p8pool = ctx.enter_context(tc.tile_pool(name="p8", bufs=2))
    opool = ctx.enter_context(tc.tile_pool(name="op", bufs=4))
    pjps = ctx.enter_context(tc.tile_pool(name="pj", bufs=3, space="PSUM"))
    scps = ctx.enter_context(tc.tile_pool(name="sc", bufs=2, space="PSUM"))
    pvps = ctx.enter_context(tc.tile_pool(name="pv", bufs=3, space="PSUM"))

    # --- persistent fp8 weights: [128, KT*C] per (tensor, hi/lo); k-tile j
    # --- lives at cols [j*C, (j+1)*C). DoubleRow views are [128, j, C].
    wtile, wview = {}, {}
    for n in WNAMES:
        for part in ("h", "l"):
            t = wpool.tile([128, KT * C], F8, tag=f"{n}{part}")
            wtile[(n, part)] = t
            wview[(n, part)] = t.rearrange("p (j c) -> p j c", c=C)

    def dma_w(n, part, j0, nj):
        src = w_d[n + part].rearrange("(j p) c -> p j c", p=128)
        nc.sync.dma_start(
            wtile[(n, part)][:, j0 * C:(j0 + nj) * C]
            .rearrange("p (j c) -> p j c", c=C),
            src[:, j0:j0 + nj, :])

    def load_x(s):
        th = xpool.tile([128, KT * LT], F8, tag="xh")
        tl = xpool.tile([128, KT * LT], F8, tag="xl")
        for t, d in ((th, xh_d), (tl, xl_d)):
            nc.sync.dma_start(
                t[:].rearrange("p (j n) -> p j n", n=LT),
                d.rearrange("(j p) l -> p j l", p=128)[:, :, s * LT:(s + 1) * LT])
        return th, tl

    # Startup DMA order matches the k-blocked strip-0 consumption exactly:
    # per j-pair (wq-hi, x-hi) for the Q hi-pass (PE starts after the first
    # two transfers), then x-lo (Q lo-pass), wq-lo, wk hi/lo (K passes),
    # then V/O weights needed later.
    xh0 = xpool.tile([128, KT * LT], F8, tag="xh")
    xl0 = xpool.tile([128, KT * LT], F8, tag="xl")

    def dma_x0jp(t, d, jp):
        nc.sync.dma_start(
            t[:, 2 * jp * LT:(2 * jp + 2) * LT]
            .rearrange("p (j n) -> p j n", n=LT),
            d.rearrange("(j p) l -> p j l", p=128)[:, 2 * jp:2 * jp + 2, 0:LT])

    for jp in range(NJP):
        dma_w("wq", "h", 2 * jp, 2)
        dma_x0jp(xh0, xh_d, jp)
    for jp in range(NJP):
        dma_x0jp(xl0, xl_d, jp)
    for jp in range(NJP):
        dma_w("wq", "l", 2 * jp, 2)
    for jp in range(NJP):
        dma_w("wk", "h", 2 * jp, 2)
    for jp in range(NJP):
        dma_w("wk", "l", 2 * jp, 2)
    for n in ("wv", "wo"):
        for part in ("h", "l"):
            if (n, part) == ("wv", "l"):
                # k-tiles 6,7 of wv-lo are never read (dropped V correction)
                dma_w(n, part, 0, NJP)
                dma_w(n, part, NJP, 2)
            else:
                for half in range(2):
                    dma_w(n, part, half * NJP, NJP)
    x_next = (xh0, xl0)

    # e_t is a single persistent buffer: exps rewrite the diagonal blocks every
    # strip, the off-diagonal stays zero from this one memset.
    e_t = epool.tile([128, H * LT], F16, tag="e")
    nc.gpsimd.memset(e_t[:], 0.0)

    def dr3(ps, pairs, lcols, rcols, jp_major=False, jp_counts=None):
        """DoubleRow matmuls into one psum group. Term-major (hi*Whi terms
        first) needs the fewest tensors before the PE can begin (startup);
        jp-major defers the last j-pair to the group's end (output drain,
        where late heads' P splits arrive last).
        pairs = ((rhs, lhsT), ...) views [128, j, *]; lcols/rcols slice the
        stationary/moving free columns. jp_counts (per pair, default NJP)
        truncates a term to its first N j-pairs - used to drop a fraction of
        a low-order correction term where the accuracy budget allows."""
        cnt = jp_counts or (NJP,) * len(pairs)
        order = ([(rv, lv, jp) for jp in range(NJP)
                  for (rv, lv), c in zip(pairs, cnt) if jp < c]
                 if jp_major else
                 [(rv, lv, jp) for (rv, lv), c in zip(pairs, cnt)
                  for jp in range(c)])
        for n, (rv, lv, jp) in enumerate(order):
            nc.tensor.matmul(ps,
                             lv[:, 2 * jp:2 * jp + 2, lcols],
                             rv[:, 2 * jp:2 * jp + 2, rcols],
                             start=(n == 0), stop=(n == len(order) - 1),
                             perf_mode=DRMODE)

    for s in range(NS):
        ls = s * LT
        xh_t, xl_t = x_next if s == 0 else x_next
        xh_v = xh_t.rearrange("p (j n) -> p j n", n=LT)
        xl_v = xl_t.rearrange("p (j n) -> p j n", n=LT)

        qk_t = qpool.tile([128, 2 * KT * LT], F16, tag="qk")
        en_t = npool.tile([128, H * LT], F16, tag="en")

        def attn_chain(h):
            # scores -> exp -> Pool column-sum -> reciprocal -> normalize
            qb = h * 2 * LT
            kb = qb + LT
            sc = scps.tile([128, LT], F32, tag="sc")
            for p in range(NP):
                nc.tensor.matmul(sc[:, p * PAIR:(p + 1) * PAIR],
                                 qk_t[:, kb + p * PAIR:kb + (p + 1) * PAIR],
                                 qk_t[:, qb + p * PAIR:qb + (p + 1) * PAIR],
                                 start=True, stop=True)
            eh = e_t[:, h * LT:(h + 1) * LT]
            for r0, c0 in ((0, 0), (64, 64)):
                nc.scalar.activation(
                    eh[r0:r0 + 64, :].rearrange("a (np c) -> a np c", c=PAIR)[:, :, c0:c0 + 64],
                    sc[r0:r0 + 64, :].rearrange("a (np c) -> a np c", c=PAIR)[:, :, c0:c0 + 64],
                    mybir.ActivationFunctionType.Exp, scale=SCALE)
            den = dpool.tile([128, LT], F32, tag="den")
            nc.gpsimd.partition_all_reduce(den[:], eh, channels=128,
                                           reduce_op=bass_isa.ReduceOp.add)
            r_t = rpool.tile([128, LT], F16, tag="r")
            with nc.allow_low_precision(reason="softmax recip fp16 ample"):
                nc.vector.reciprocal(r_t[:], den[:])
            nc.vector.tensor_mul(en_t[:, h * LT:(h + 1) * LT], eh, r_t[:])

        def o_group(ph_v, pl_v, lsp, m, halves=1):
            # halves=2 splits the group into two 256-col psum groups so the
            # first half's evict+DMA overlaps the second half's matmuls
            # (used for the very last output block to shorten the drain).
            woh, wol = wview[("wo", "h")], wview[("wo", "l")]
            hw = LT // halves
            for hf in range(halves):
                cs = slice(hf * hw, (hf + 1) * hw)
                ps = pjps.tile([128, 512], F32, tag="pj")
                dr3(ps[:, 0:hw], ((ph_v[:, :, cs], woh), (pl_v[:, :, cs], woh),
                                  (ph_v[:, :, cs], wol)),
                    slice(m * 128, (m + 1) * 128), slice(None),
                    jp_major=True)
                o_t = opool.tile([128, hw], F32, tag=f"o{hf}" if halves > 1 else "o")
                nc.vector.tensor_scalar_mul(o_t[:], ps[:, 0:hw], OSCALE)
                nc.sync.dma_start(
                    o_d[m * 128:(m + 1) * 128, lsp + hf * hw:lsp + (hf + 1) * hw],
                    o_t[:, 0:hw])

        # --- Q/K projections (fp8 DoubleRow); the softmax chain for head h
        # --- is emitted after Q/K of head h+2 so the PE never waits on the
        # --- ACT evictions feeding the scores matmuls.
        if s == 0:
            # strip 0 is DMA-paced: k-block 4 heads per pass, term-major, so
            # each (w j-pair, x j-pair) transfer feeds 4 heads' matmuls the
            # moment it lands and the hi-pass starts after just two DMAs.
            def qk_block(nm, off_base, h0):
                wh, wl = wview[(nm, "h")], wview[(nm, "l")]
                # 4 concurrent banks: 3 from pjps + 1 borrowed from pvps
                # (pv psums are idle during the strip-0 Q/K phase)
                pss = [pjps.tile([128, 512], F32, tag="pj", name=f"qkps{i}")
                       for i in range(3)]
                pss.append(pvps.tile([128, NP * PAIR], F32, tag="pv",
                                     name="qkps3"))
                for ti, (rv, lv) in enumerate(((xh_v, wh), (xl_v, wh))):
                    for jp in range(NJP):
                        for hi in range(4):
                            h = h0 + hi
                            nc.tensor.matmul(
                                pss[hi][:, 0:LT],
                                lv[:, 2 * jp:2 * jp + 2, h * 128:(h + 1) * 128],
                                rv[:, 2 * jp:2 * jp + 2, :],
                                start=(ti == 0 and jp == 0), stop=False,
                                perf_mode=DRMODE)
                # last term head-outer: bank hi closes (and evicts) while
                # bank hi+1 is still accumulating, so the next block's first
                # psum never waits on a burst of 4 back-to-back evictions
                for hi in range(4):
                    h = h0 + hi
                    for jp in range(NJP):
                        nc.tensor.matmul(
                            pss[hi][:, 0:LT],
                            wl[:, 2 * jp:2 * jp + 2, h * 128:(h + 1) * 128],
                            xh_v[:, 2 * jp:2 * jp + 2, :],
                            start=False, stop=(jp == NJP - 1),
                            perf_mode=DRMODE)
                    off = off_base + h * 2 * LT
                    nc.scalar.activation(qk_t[:, off:off + LT],
                                         pss[hi][:, 0:LT], ACOPY, scale=QSCALE)

            qk_block("wq", 0, 0)
            qk_block("wq", 0, 4)
            qk_block("wk", LT, 0)
            for h in range(4):
                attn_chain(h)
            qk_block("wk", LT, 4)
            attn_chain(4)
            attn_chain(5)
        else:
            for h in range(H):
                qb = h * 2 * LT
                kb = qb + LT
                for off, nm in ((qb, "wq"), (kb, "wk")):
                    ps = pjps.tile([128, 512], F32, tag="pj")
                    wh, wl = wview[(nm, "h")], wview[(nm, "l")]
                    dr3(ps[:, 0:LT], ((xh_v, wh), (xl_v, wh), (xh_v, wl)),
                        slice(h * 128, (h + 1) * 128), slice(None))
                    nc.scalar.activation(qk_t[:, off:off + LT], ps[:, 0:LT],
                                         ACOPY, scale=QSCALE)
                if h >= 3:
                    attn_chain(h - 3)

        # prefetch next strip's x now, BEFORE the output-projection DMAs are
        # queued, so the next strip's first matmul never waits on the queue
        if s + 1 < NS:
            x_next = load_x(s + 1)

        # --- output projection of the PREVIOUS strip (fp8 DoubleRow over the
        # --- split P). Pure-runway PE work; the two remaining softmax chains
        # --- interleave between its groups.
        om = 0

        def o_drain(k):
            nonlocal om
            while om < k:
                o_group(*p_prev, ls_prev, om)
                om += 1

        # --- V projection (token-major, fp8 DoubleRow): V[l, c]. On strip 0
        # --- all softmax chains are still pending (the split startup passes
        # --- left no room); interleave one per V group so they pace on the
        # --- side engines under the V matmuls.
        v_t = vpool.tile([128, NP * C], F16, tag="v")
        wvh, wvl = wview[("wv", "h")], wview[("wv", "l")]
        ph8_t = p8pool.tile([128, KT * LT], F8, tag="ph8")
        pl8_t = p8pool.tile([128, KT * LT], F8, tag="pl8")

        def v_group(p, n2, halves=1):
            # halves=2 on the LAST group: its final eviction shrinks to 256
            # cols and lands earlier, so the first pv_head's Ldweights (whose
            # sem wait batches up to the last v eviction) never stalls
            hw = LT // halves
            for hf in range(halves):
                cs = slice(n2 * LT + hf * hw, n2 * LT + (hf + 1) * hw)
                ps = pjps.tile([128, 512], F32, tag="pj")
                # V's hi*Wlo correction drops its last j-pair (k-tiles 6,7):
                # +1.1e-2 rel err (measured, vs 2e-2 gate) for -2048 PE
                # cycles per strip
                dr3(ps[:, 0:hw], ((wvh, xh_v), (wvh, xl_v), (wvl, xh_v)),
                    slice(p * 128, (p + 1) * 128), cs,
                    jp_counts=(NJP, NJP, NJP - 1))
                nc.scalar.activation(
                    v_t[:, p * C + n2 * LT + hf * hw:
                        p * C + n2 * LT + (hf + 1) * hw], ps[:, 0:hw],
                    ACOPY, scale=QSCALE)

        def pv_head(h):
            # P^T[d, q]; hi/lo fp8 split directly off PSUM on two fast
            # engines: ph8 = fp8(SP*P) (ACT Copy with scale), then
            # pl8 = (SP*P) - ph8 in one DVE scalar_tensor_tensor. Keeps the
            # split off Pool so the o_proj never stalls on a serial chain.
            pv = pvps.tile([128, NP * PAIR], F32, tag="pv")
            for p in range(NP):
                nc.tensor.matmul(pv[:, p * PAIR:(p + 1) * PAIR],
                                 v_t[:, p * C + h * 128:p * C + (h + 1) * 128],
                                 en_t[:, h * LT + p * PAIR:h * LT + (p + 1) * PAIR],
                                 start=True, stop=True)
            hs = slice(h * LT, (h + 1) * LT)
            with nc.allow_low_precision(reason="fp8 split, compensated"):
                nc.scalar.activation(ph8_t[:, hs], pv[:], ACOPY, scale=SP)
                # NOTE: must stay on ACT/DVE - GPSIMD cannot read PSUM (BIR
                # verifier rejects it; CoreSim does not catch this).
                nc.vector.scalar_tensor_tensor(
                    pl8_t[:, hs], pv[:], SP, ph8_t[:, hs],
                    mybir.AluOpType.mult, mybir.AluOpType.subtract)

        if s == NS - 1 and s > 0:
            # final strip: run the remaining softmax chains and the whole
            # V/PV/split block BEFORE the previous strip's output projection,
            # so every P split lands while the PE chews through o_drain(KT)
            # (pure runway) and the last o_proj never waits on a split.
            attn_chain(H - 3)
            attn_chain(H - 2)
            attn_chain(H - 1)
            for n2 in range(C // LT):
                for p in range(NP):
                    v_group(p, n2, halves=2 if p == NP - 1 else 1)
                for h in range(4 * n2, 4 * n2 + 4):
                    pv_head(h)
            o_drain(KT)
        else:
            if s >= 1:
                o_drain(2)
            if s > 0:
                attn_chain(H - 3)
            if s >= 1:
                o_drain(4)
            attn_chain(H - 2)
            if s >= 1:
                o_drain(6)
            attn_chain(H - 1)
            if s >= 1:
                o_drain(KT)
            for p in range(NP):
                for n2 in range(C // LT):
                    v_group(p, n2, halves=2 if (p, n2) == (NP - 1, 1) else 1)
            for h in range(H):
                pv_head(h)

        p_prev = (ph8_t.rearrange("p (j n) -> p j n", n=LT),
                  pl8_t.rearrange("p (j n) -> p j n", n=LT))
        ls_prev = ls
    # final strip's output projection. The last block splits unevenly
    # (384+128): the big first part's evict+DMA overlaps the small second
    # part's matmuls, and the very last DMA is a quarter-size transfer.
    for m in range(KT - 1):
        o_group(*p_prev, ls_prev, m)
    ph_v, pl_v = p_prev
    woh, wol = wview[("wo", "h")], wview[("wo", "l")]
    m = KT - 1
    for c0, cw, tg in ((0, 384, "oa"), (384, 128, "ob")):
        cs = slice(c0, c0 + cw)
        ps = pjps.tile([128, 512], F32, tag="pj", name="olast")
        dr3(ps[:, 0:cw], ((ph_v[:, :, cs], woh), (pl_v[:, :, cs], woh),
                          (ph_v[:, :, cs], wol)),
            slice(m * 128, (m + 1) * 128), slice(None), jp_major=True)
        o_t = opool.tile([128, cw], F32, tag=tg, name=f"o_{tg}")
        nc.vector.tensor_scalar_mul(o_t[:], ps[:, 0:cw], OSCALE)
        nc.sync.dma_start(
            o_d[m * 128:(m + 1) * 128, ls_prev + c0:ls_prev + c0 + cw],
            o_t[:, 0:cw])


def build_nc(l_total=L):
    nc = bacc.Bacc("TRN2", target_bir_lowering=False, debug=False,
                   enable_asserts=False)
    xh_d = nc.dram_tensor("xh", [C, l_total], F8, kind="ExternalInput").ap()
    xl_d = nc.dram_tensor("xl", [C, l_total], F8, kind="ExternalInput").ap()
    w_d = {}
    for n in WNAMES:
        for part in ("h", "l"):
            w_d[n + part] = nc.dram_tensor(n + part, [C, C], F8,
                                           kind="ExternalInput").ap()
    o_d = nc.dram_tensor("out", [C, l_total], F32, kind="ExternalOutput").ap()
    with tile.TileContext(nc) as tc:
        with ExitStack() as ctx:
            _emit(ctx, tc, xh_d, xl_d, w_d, o_d, l_total)
    nc.compile()
    return nc


_NC_CACHE = {}


def _get_nc(l_total):
    if l_total not in _NC_CACHE:
        _NC_CACHE[l_total] = build_nc(l_total)
    return _NC_CACHE[l_total]


def _split8(a, scale):
    s = np.asarray(a, np.float32) * scale
    hi = s.astype(NP8)
    lo = (s - hi.astype(np.float32)).astype(NP8)
    return np.ascontiguousarray(hi), np.ascontiguousarray(lo)


def make_in_maps(x, Wq, Wk, Wv, Wo):
    ws = {}
    for n, w in zip(WNAMES, (Wq, Wk, Wv, Wo)):
        ws[n + "h"], ws[n + "l"] = _split8(w, SW)
    xs = np.asarray(x, np.float32)
    in_maps = []
    for i in range(x.shape[0]):
        xh, xl = _split8(xs[i], SX)
        m = {"xh": xh, "xl": xl}
        m.update(ws)
        in_maps.append(m)
    return in_maps


def _numpy_fallback(x, Wq, bq, Wk, bk, Wv, bv, Wo, bo):
    # Exact host-side path, used only if biases are nonzero (the problem spec
    # fills them with zeros, so the device kernel does not apply them).
    x = np.asarray(x, np.float32)
    Bn, Cn, Ln = x.shape
    hd = Cn // H
    nch = Ln // CHUNK
    xt = np.transpose(x, (0, 2, 1))
    Q = (xt @ Wq + bq).reshape(Bn, nch, CHUNK, H, hd)
    K = (xt @ Wk + bk).reshape(Bn, nch, CHUNK, H, hd)
    V = (xt @ Wv + bv).reshape(Bn, nch, CHUNK, H, hd)
    scores = np.einsum("bnqhd,bnkhd->bnhqk", Q, K) / np.sqrt(hd)
    scores -= scores.max(axis=-1, keepdims=True)
    e = np.exp(scores)
    attn = e / e.sum(axis=-1, keepdims=True)
    out = np.einsum("bnhqk,bnkhd->bnqhd", attn, V).reshape(Bn, Ln, Cn)
    out = out @ Wo + bo
    return np.ascontiguousarray(np.transpose(out, (0, 2, 1)).astype(np.float32))


def kernel(x, Wq, bq, Wk, bk, Wv, bv, Wo, bo, trace=False):
    from concourse.bass_utils import run_bass_kernel_spmd
    nb, c_in, l_total = x.shape
    if (any(np.any(np.asarray(b) != 0) for b in (bq, bk, bv, bo))
            or c_in != C or l_total % LT != 0 or nb > N_CORES):
        return _numpy_fallback(x, Wq, bq, Wk, bk, Wv, bv, Wo, bo)
    nc = _get_nc(l_total)
    in_maps = make_in_maps(x, Wq, Wk, Wv, Wo)
    res = run_bass_kernel_spmd(nc, in_maps, core_ids=list(range(nb)), trace=trace)
    out = np.stack([res.results[i]["out"] for i in range(nb)], axis=0)
    if trace:
        return out, res
    return out

